# revision 15
# baseline (speedup 1.0000x reference)
"""Trainium2 Bass kernel for nn_ReaReaConv (GCN-style message passing with
dynamic edge gating) — bf16 redesign.

Math (per batch b):
    deg[n]   = in-degree(n) + 1 (self loop);  dis = rsqrt(deg)
    f_e      = keep*fdo + (1-keep)*(1-fdo), keep = sigmoid(2*flux[src]*flux[tgt])
    out[t]   = dis_t * ( Wc @ (T-V)[t] + Wd @ V[t] ) + bias
    T[t]     = sum_{e->t} dis_src * x[src_e]          (self loop: f=0 edge)
    V[t]     = sum_{e->t} dis_src * f_e * x[src_e]

Design:
 - Targets are bin-packed (host, integer-only) into 800 tiles of 64 targets
   each, balancing per-tile A/B edge counts so chunk capacities stay tight.
   Output rows are permuted back on the host.
 - Gather tables are bf16 with batch-interleaved rows (x[n] = [f0b0, f0b1,
   f1b0, f1b1, ...], 256B rows) split A/B at 32768 for int16 indices.
 - Per 4-tile group: 2 dma_gathers (A/B tables), one merged is_equal builds
   all chunks' one-hots (pair-duplicated tl/iota for DVE 2x bf16 mode), one
   merged multiply scales by dis_src, one merged multiply builds f-scaled
   V-operands (f0/f1 pairs ride the batch-interleave).
 - Matmuls: one-hot chunk is the stationary operand ([128 edges, 64 tgts]);
   raw x and f-scaled x are the moving operands, accumulating T/V in a
   [128, 128] PSUM tile holding a PAIR of 64-target tiles via col strips
   (tile_position=(0, 0|64)).
 - Epilogue per pair: um/vm -> PE transpose -> projection with interleaved
   block weights W2 (also de-interleaves batches) -> dis_tgt scale + bias.
 - Trailing pad slots use index -1 (Q7 trims them: no DMA descriptors).
"""

import heapq

import numpy as np
import ml_dtypes

BF16 = ml_dtypes.bfloat16

# -------------------- problem constants --------------------
N_NODES = 50000
N_EDGES = 1600000
BATCH = 2
C = 64            # per-batch channels
IC = 2 * C        # interleaved feature width (both batches)
N_CORES = 8
T = 64            # targets per tile
TPG = 4           # tiles per gather group
NT_CORE = 100     # tiles per core (incl. pad tiles)
NG = NT_CORE // TPG
NPAIR = NT_CORE // 2
N_TILES = N_CORES * NT_CORE   # 800
CHUNK = 128
SPLIT = 32768
SELF_FLUX = 30.0  # sigmoid(2*30*30)==1.0 -> f==0 for self-loop edges
PAD_NEG1 = False  # -1 trailing-pad idxs fault on deployed HW ucode; keep 0-pads
# per-group SWDGE queue assignment for the A/B gathers. The tile scheduler
# may reorder gathers, and each DMASW sem lane (scheduled order % 8) is
# locked to one queue -- a single queue is always consistent.
QA = [0] * NG
QB = [0] * NG


class Cfg(tuple):
    """(capa, capb) chunk capacities per tile for the A/B tables."""
    @property
    def capa(self):
        return self[0]

    @property
    def capb(self):
        return self[1]

    @property
    def ct(self):
        return self[0] + self[1]

    @property
    def ctg(self):          # chunks per group
        return TPG * self.ct

    @property
    def ctn(self):          # chunks per core
        return NG * self.ctg

    @property
    def na(self):
        return min(SPLIT, N_NODES)

    @property
    def nb(self):
        return N_NODES - self.na


# -------------------- host prep (indices / layout only) --------------------

def _wrap16(idx_flat):
    """dma_gather index layout: [128, n/16] int16, idx[p, s] = flat[s*16+p],
    replicated across the 8 gpsimd cores (partition blocks of 16)."""
    n = len(idx_flat)
    assert n % 16 == 0
    w = np.asarray(idx_flat, np.int16).reshape(n // 16, 16).T  # [16, n/16]
    return np.tile(w, (8, 1))  # [128, n/16]


def _assign_tiles(degA, degB):
    """Greedy 2D bin-packing of targets into N_TILES tiles of <=T targets,
    balancing per-tile A and B edge counts. Integer/layout work only."""
    deg = degA + degB
    order = np.argsort(-deg, kind="stable")
    avgA = max(float(degA.sum()) / N_TILES, 1.0)
    avgB = max(float(degB.sum()) / N_TILES, 1.0)
    loadA = np.zeros(N_TILES, np.int64)
    loadB = np.zeros(N_TILES, np.int64)
    counts = np.zeros(N_TILES, np.int32)
    heap = [(0.0, i) for i in range(N_TILES)]
    heapq.heapify(heap)
    tile_of = np.empty(N_NODES, np.int32)
    slot_of = np.empty(N_NODES, np.int32)
    for t in order:
        while True:
            key, i = heapq.heappop(heap)
            if counts[i] < T:
                break
        tile_of[t] = i
        slot_of[t] = counts[i]
        counts[i] += 1
        loadA[i] += degA[t]
        loadB[i] += degB[t]
        if counts[i] < T:
            heapq.heappush(
                heap, (max(loadA[i] / avgA, loadB[i] / avgB), i))
    return tile_of, slot_of


def prep(x, edge_index, f_disc_orig, fluxes):
    """Returns (cfg, shared dict, per-core dicts, target_rows). Integer /
    index / layout / dtype work only — no floating-point arithmetic."""
    n = N_NODES
    src0 = np.asarray(edge_index[0]).astype(np.int64)
    tgt0 = np.asarray(edge_index[1]).astype(np.int64)
    x = np.asarray(x, np.float32)
    fdo_in = np.asarray(f_disc_orig, np.float32)
    fluxes = np.asarray(fluxes, np.float32)

    deg = (np.bincount(tgt0, minlength=n) + 1).astype(np.int64)

    # per-target A/B in-edge counts (self loop included)
    isA0 = src0 < SPLIT
    degA = np.bincount(tgt0[isA0], minlength=n)
    degA += (np.arange(n) < SPLIT)
    degB = deg - degA

    tile_of, slot_of = _assign_tiles(degA, degB)

    # all edges incl self loops
    loops = np.arange(n, dtype=np.int64)
    src_all = np.concatenate([src0, loops])
    tgt_all = np.concatenate([tgt0, loops])
    sf = np.full(n, SELF_FLUX, np.float32)
    per_edge = np.stack([
        np.concatenate([fdo_in, np.zeros(n, np.float32)]),
        np.concatenate([fluxes[0][src0], sf]),
        np.concatenate([fluxes[1][src0], sf]),
        np.concatenate([fluxes[0][tgt0], sf]),
        np.concatenate([fluxes[1][tgt0], sf]),
        deg[src_all].astype(np.float32),
    ])  # [6, E+N]: fdo, fs0, fs1, ft0, ft1, degs

    tid = tile_of[tgt_all].astype(np.int64)
    table = (src_all >= SPLIT).astype(np.int64)
    bucket = tid * 2 + table
    order = np.argsort(bucket, kind="stable")
    counts = np.bincount(bucket, minlength=N_TILES * 2)
    starts = np.concatenate([[0], np.cumsum(counts)])
    rank = np.arange(len(order)) - np.repeat(starts[:-1], counts)

    capa = int(-(-counts[0::2].max() // CHUNK))
    capb = int(-(-counts[1::2].max() // CHUNK))
    cfg = Cfg((capa, capb))
    ct, ctg, ctn = cfg.ct, cfg.ctg, cfg.ctn

    # within-core chunk base per (tile, table)
    t_local = np.arange(N_TILES) % NT_CORE
    g_of = t_local // TPG
    k_of = t_local % TPG
    baseA = g_of * ctg + k_of * capa
    baseB = g_of * ctg + TPG * capa + k_of * capb
    base_chunk = np.empty(N_TILES * 2, np.int64)
    base_chunk[0::2] = baseA
    base_chunk[1::2] = baseB

    eo = order
    cc = base_chunk[bucket[eo]] + rank // CHUNK     # within-core chunk
    pp = rank % CHUNK                               # partition
    co = tid[eo] // NT_CORE                         # core

    tl = np.full((N_CORES, 128, ctn), -1.0, np.float32)
    meta = np.zeros((N_CORES, 6, 128, ctn), np.float32)
    meta[:, 1:5] = SELF_FLUX    # pad fs/ft -> keep=1, fdo=0 -> f=0
    meta[:, 5] = 1.0            # pad deg_src = 1
    idxflat = np.zeros((N_CORES, ctn * CHUNK), np.int64)
    realm = np.zeros((N_CORES, ctn * CHUNK), bool)

    tl[co, pp, cc] = slot_of[tgt_all[eo]].astype(np.float32)
    for j in range(6):
        meta[co, j, pp, cc] = per_edge[j][eo]
    src_adj = src_all[eo] - table[eo] * cfg.na
    idxflat[co, cc * CHUNK + pp] = src_adj
    realm[co, cc * CHUNK + pp] = True

    # trailing -1 per gather call; wrap16 per call
    cores = []
    for core in range(N_CORES):
        blocks = []
        for g in range(NG):
            a0 = (g * ctg) * CHUNK
            a1 = (g * ctg + TPG * capa) * CHUNK
            b1 = ((g + 1) * ctg) * CHUNK
            for (s, e) in ((a0, a1), (a1, b1)):
                seg = idxflat[core, s:e].copy()
                if PAD_NEG1:
                    rm = realm[core, s:e]
                    nz = np.nonzero(rm)[0]
                    last = nz[-1] if len(nz) else -1
                    seg[last + 1:] = -1
                blocks.append(_wrap16(seg))
        idx16 = np.concatenate(blocks, axis=1)

        tl2 = np.repeat(tl[core], 2, axis=1).astype(BF16)  # [128, ctn*2]
        d = {
            "idx16": idx16,
            "tl2": np.ascontiguousarray(tl2),
        }
        for j, nm in enumerate(["fdo", "fs0", "fs1", "ft0", "ft1", "degs"]):
            d[nm] = np.ascontiguousarray(meta[core, j])
        cores.append(d)

    # target id per output row: row = tile_local*T + slot
    target_rows = np.full((N_CORES, NT_CORE * T), -1, np.int64)
    target_rows[tile_of // NT_CORE, (tile_of % NT_CORE) * T + slot_of] = \
        np.arange(n)

    # degown: [128, NPAIR], partition = row within pair
    for core in range(N_CORES):
        tr = target_rows[core]
        dg = np.ones(NT_CORE * T, np.float32)
        valid = tr >= 0
        dg[valid] = deg[tr[valid]].astype(np.float32)
        cores[core]["degown"] = np.ascontiguousarray(
            dg.reshape(NPAIR, 128).T)

    # shared gather tables: batch-interleaved bf16 rows
    xp = np.empty((n, IC), np.float32)
    xp[:, 0::2] = x[0]
    xp[:, 1::2] = x[1]
    xp16 = xp.astype(BF16)
    shared = {
        "xpa": np.ascontiguousarray(xp16[:cfg.na]),
        "xpb": np.ascontiguousarray(xp16[cfg.na:]),
        "iota2": np.tile(np.arange(T, dtype=np.float32).astype(BF16),
                         (128, 1)),
        "ident": np.eye(128, dtype=np.float32).astype(BF16),
    }
    return cfg, shared, cores, target_rows


def _shared_weights(W_conc, W_disc, bias):
    """W2[2j+b, o+64b] = W[o, j]; bias2 replicated (fp32)."""
    Wc = np.asarray(W_conc, np.float32)
    Wd = np.asarray(W_disc, np.float32)
    w2c = np.zeros((128, IC), np.float32)
    w2d = np.zeros((128, IC), np.float32)
    j = np.arange(C)
    o = np.arange(C)
    for b in range(2):
        w2c[np.ix_(2 * j + b, o + C * b)] = Wc.T
        w2d[np.ix_(2 * j + b, o + C * b)] = Wd.T
    bias2 = np.zeros((128, IC), np.float32)
    bias2[:, :C] = np.asarray(bias, np.float32)[None, :]
    bias2[:, C:] = np.asarray(bias, np.float32)[None, :]
    return w2c.astype(BF16), w2d.astype(BF16), bias2


# -------------------- device program --------------------

def build_nc(cfg: Cfg):
    import concourse.bass as bass  # noqa: F401
    import concourse.tile as tile
    from concourse import bacc, mybir

    dt = mybir.dt
    act = mybir.ActivationFunctionType
    alu = mybir.AluOpType

    capa, capb, ct, ctg, ctn = cfg.capa, cfg.capb, cfg.ct, cfg.ctg, cfg.ctn
    cag, cbg = TPG * capa, TPG * capb

    nc = bacc.Bacc("TRN2", target_bir_lowering=False, debug=False,
                   num_swdge_queues=4)

    xpa = nc.dram_tensor("xpa", [cfg.na, IC], dt.bfloat16, kind="ExternalInput")
    xpb = nc.dram_tensor("xpb", [cfg.nb, IC], dt.bfloat16, kind="ExternalInput")
    idx16_d = nc.dram_tensor("idx16", [128, ctn * 8], dt.int16,
                             kind="ExternalInput")
    tl2_d = nc.dram_tensor("tl2", [128, ctn * 2], dt.bfloat16,
                           kind="ExternalInput")
    fdo_d = nc.dram_tensor("fdo", [128, ctn], dt.float32, kind="ExternalInput")
    fs0_d = nc.dram_tensor("fs0", [128, ctn], dt.float32, kind="ExternalInput")
    fs1_d = nc.dram_tensor("fs1", [128, ctn], dt.float32, kind="ExternalInput")
    ft0_d = nc.dram_tensor("ft0", [128, ctn], dt.float32, kind="ExternalInput")
    ft1_d = nc.dram_tensor("ft1", [128, ctn], dt.float32, kind="ExternalInput")
    degs_d = nc.dram_tensor("degs", [128, ctn], dt.float32,
                            kind="ExternalInput")
    degown_d = nc.dram_tensor("degown", [128, NPAIR], dt.float32,
                              kind="ExternalInput")
    iota2_d = nc.dram_tensor("iota2", [128, T], dt.bfloat16,
                             kind="ExternalInput")
    ident_d = nc.dram_tensor("ident", [128, 128], dt.bfloat16,
                             kind="ExternalInput")
    w2c_d = nc.dram_tensor("w2c", [128, IC], dt.bfloat16, kind="ExternalInput")
    w2d_d = nc.dram_tensor("w2d", [128, IC], dt.bfloat16, kind="ExternalInput")
    bias2_d = nc.dram_tensor("bias2", [128, IC], dt.float32,
                             kind="ExternalInput")
    out_d = nc.dram_tensor("out", [NT_CORE * T, IC], dt.float32,
                           kind="ExternalOutput")

    with tile.TileContext(nc) as tc:
        with (
            tc.tile_pool(name="const", bufs=1) as constp,
            tc.tile_pool(name="res", bufs=1) as resp,
        ):
            iota_sb = constp.tile([128, T], dt.bfloat16)
            nc.sync.dma_start(iota_sb[:], iota2_d[:, :])
            ident_sb = constp.tile([128, 128], dt.bfloat16)
            nc.sync.dma_start(ident_sb[:], ident_d[:, :])
            w2c_sb = constp.tile([128, IC], dt.bfloat16)
            nc.sync.dma_start(w2c_sb[:], w2c_d[:, :])
            w2d_sb = constp.tile([128, IC], dt.bfloat16)
            nc.sync.dma_start(w2d_sb[:], w2d_d[:, :])
            bias_sb = constp.tile([128, IC], dt.float32)
            nc.sync.dma_start(bias_sb[:], bias2_d[:, :])

            idx_sb = resp.tile([128, ctn * 8], dt.int16)
            nc.sync.dma_start(idx_sb[:], idx16_d[:, :])
            tl2_sb = resp.tile([128, ctn * 2], dt.bfloat16)
            nc.sync.dma_start(tl2_sb[:], tl2_d[:, :])
            g2_sb = resp.tile([128, ctn * 2], dt.bfloat16)
            f01_sb = resp.tile([128, ctn * 2], dt.bfloat16)
            disown_sb = resp.tile([128, NPAIR], dt.float32)
            nc.sync.dma_start(disown_sb[:], degown_d[:, :])
            nc.vector.reciprocal(disown_sb[:], disown_sb[:])
            nc.scalar.activation(disown_sb[:], disown_sb[:], act.Sqrt)

            # ---- prepass: g2 (dis_src pairs) and f01 (f0/f1 pairs) ----
            with tc.tile_pool(name="pp", bufs=1) as ppp:
                g_sb = ppp.tile([128, ctn], dt.float32)
                nc.sync.dma_start(g_sb[:], degs_d[:, :])
                nc.vector.reciprocal(g_sb[:], g_sb[:])
                nc.scalar.activation(g_sb[:], g_sb[:], act.Sqrt)
                g2v = g2_sb[:].rearrange("p (c two) -> p c two", two=2)
                nc.vector.tensor_copy(out=g2v[:, :, 0], in_=g_sb[:])
                nc.vector.tensor_copy(out=g2v[:, :, 1], in_=g_sb[:])

                fdo_sb = ppp.tile([128, ctn], dt.float32)
                nc.sync.dma_start(fdo_sb[:], fdo_d[:, :])
                c1 = ppp.tile([128, ctn], dt.float32)
                nc.vector.tensor_scalar(
                    c1[:], fdo_sb[:], 2.0, -1.0, alu.mult, alu.add)
                c0 = ppp.tile([128, ctn], dt.float32)
                nc.vector.tensor_scalar(
                    c0[:], fdo_sb[:], -1.0, 1.0, alu.mult, alu.add)
                f01v = f01_sb[:].rearrange("p (c two) -> p c two", two=2)
                for b, (fsd, ftd) in enumerate(((fs0_d, ft0_d),
                                                (fs1_d, ft1_d))):
                    fs_sb = ppp.tile([128, ctn], dt.float32, tag="fs")
                    nc.sync.dma_start(fs_sb[:], fsd[:, :])
                    ft_sb = ppp.tile([128, ctn], dt.float32, tag="ft")
                    nc.sync.dma_start(ft_sb[:], ftd[:, :])
                    nc.vector.tensor_mul(fs_sb[:], fs_sb[:], ft_sb[:])
                    nc.scalar.activation(
                        ft_sb[:], fs_sb[:], act.Sigmoid, scale=2.0)
                    nc.vector.tensor_mul(ft_sb[:], ft_sb[:], c1[:])
                    nc.vector.tensor_add(ft_sb[:], ft_sb[:], c0[:])
                    nc.vector.tensor_copy(out=f01v[:, :, b], in_=ft_sb[:])

            # ---- main loop over gather groups ----
            with (
                tc.tile_pool(name="xg", bufs=4) as xgp,
                tc.tile_pool(name="oh", bufs=2) as ohp,
                tc.tile_pool(name="wv", bufs=2) as wvp,
                tc.tile_pool(name="uv", bufs=2) as uvp,
                tc.tile_pool(name="uvt", bufs=2) as uvtp,
                tc.tile_pool(name="outp", bufs=2) as outsp,
                tc.tile_pool(name="ps_tv", bufs=2, space="PSUM") as pstv,
                tc.tile_pool(name="ps_tr", bufs=2, space="PSUM") as pstr,
                tc.tile_pool(name="ps_o", bufs=2, space="PSUM") as pso,
            ):
                tl2v = tl2_sb[:].rearrange("p (c two) -> p c two", two=2)
                g2v = g2_sb[:].rearrange("p (c two) -> p c two", two=2)
                f01v = f01_sb[:].rearrange("p (c two) -> p c two", two=2)
                iota4 = (iota_sb[:]
                         .rearrange("p (a b) -> p a b", b=2)
                         .unsqueeze(1))
                for g in range(NG):
                    xg = xgp.tile([128, ctg * IC], dt.bfloat16, tag="xg")
                    if g < 4:
                        nc.vector.memset(xg[:], 0.0)
                    xg3 = xg[:].rearrange("p (c r) -> p c r", r=IC)
                    ib = g * ctg * 8
                    nc.gpsimd.dma_gather(
                        xg3[:, 0:cag], xpa[:, :],
                        idx_sb[:, ib: ib + cag * 8],
                        cag * CHUNK, cag * CHUNK, IC,
                        single_packet=False, queue_num=QA[g],
                    )
                    nc.gpsimd.dma_gather(
                        xg3[:, cag:ctg], xpb[:, :],
                        idx_sb[:, ib + cag * 8: ib + ctg * 8],
                        cbg * CHUNK, cbg * CHUNK, IC,
                        single_packet=False, queue_num=QB[g],
                    )

                    gc = slice(g * ctg, (g + 1) * ctg)
                    # one-hot: is_equal then *dis_src (pair-packed, 2x mode)
                    o_all = ohp.tile([128, ctg * T], dt.bfloat16, tag="oh")
                    o4 = o_all[:].rearrange("p (c a b) -> p c a b", a=T // 2,
                                            b=2)
                    nc.vector.tensor_tensor(
                        o4,
                        tl2v[:, gc, :].unsqueeze(2)
                        .to_broadcast([128, ctg, T // 2, 2]),
                        iota4.to_broadcast([128, ctg, T // 2, 2]),
                        alu.is_equal,
                    )
                    nc.vector.tensor_tensor(
                        o4, o4,
                        g2v[:, gc, :].unsqueeze(2)
                        .to_broadcast([128, ctg, T // 2, 2]),
                        alu.mult,
                    )
                    # f-scaled V operand (f0/f1 pairs ride batch interleave)
                    wv = wvp.tile([128, ctg * IC], dt.bfloat16, tag="wv")
                    wv4 = wv[:].rearrange("p (c a b) -> p c a b", a=C, b=2)
                    xg4 = xg[:].rearrange("p (c a b) -> p c a b", a=C, b=2)
                    nc.vector.tensor_tensor(
                        wv4, xg4,
                        f01v[:, gc, :].unsqueeze(2)
                        .to_broadcast([128, ctg, C, 2]),
                        alu.mult,
                    )

                    o3 = o_all[:].rearrange("p (c t) -> p c t", t=T)
                    wv3 = wv[:].rearrange("p (c r) -> p c r", r=IC)
                    for q in range(2):
                        t_tile = pstv.tile([128, IC], dt.float32,
                                           tag="t_ps", name="t_tile")
                        v_tile = pstv.tile([128, IC], dt.float32,
                                           tag="v_ps", name="v_tile")
                        t_ps = t_tile[:, :]
                        v_ps = v_tile[:, :]
                        for ci in range(ct):
                            for s in range(2):
                                k = 2 * q + s
                                if ci < capa:
                                    c = k * capa + ci
                                else:
                                    c = cag + k * capb + (ci - capa)
                                lhs = o3[:, c, :]
                                rows = slice(64 * s, 64 * s + 64)
                                nc.tensor.matmul(
                                    out=t_ps[rows, :], lhsT=lhs,
                                    rhs=xg3[:, c, :],
                                    start=(ci == 0), stop=(ci == ct - 1),
                                    tile_position=(0, 64 * s),
                                    skip_group_check=True,
                                )
                                nc.tensor.matmul(
                                    out=v_ps[rows, :], lhsT=lhs,
                                    rhs=wv3[:, c, :],
                                    start=(ci == 0), stop=(ci == ct - 1),
                                    tile_position=(0, 64 * s),
                                    skip_group_check=True,
                                )

                        # epilogue for this pair of tiles
                        pid = g * 2 + q
                        vm = uvp.tile([128, IC], dt.bfloat16, tag="vm")
                        nc.vector.tensor_copy(out=vm[:], in_=v_ps)
                        um = uvp.tile([128, IC], dt.bfloat16, tag="um")
                        nc.vector.tensor_tensor(
                            um[:], t_ps, vm[:], alu.subtract)
                        tr_ps = pstr.tile([128, 256], dt.bfloat16,
                                          tag="tr")
                        nc.tensor.transpose(
                            tr_ps[:, 0:128], um[:], ident_sb[:])
                        nc.tensor.transpose(
                            tr_ps[:, 128:256], vm[:], ident_sb[:])
                        uvt = uvtp.tile([128, 256], dt.bfloat16, tag="uvt")
                        nc.vector.tensor_copy(out=uvt[:], in_=tr_ps[:])
                        umt = uvt[:, 0:128]
                        vmt = uvt[:, 128:256]
                        op_ps = pso.tile([128, IC], dt.float32, tag="op")
                        nc.tensor.matmul(
                            out=op_ps[:], lhsT=umt, rhs=w2c_sb[:],
                            start=True, stop=False)
                        nc.tensor.matmul(
                            out=op_ps[:], lhsT=vmt, rhs=w2d_sb[:],
                            start=False, stop=True)
                        o_sb = outsp.tile([128, IC], dt.float32, tag="os")
                        nc.vector.tensor_scalar(
                            o_sb[:], op_ps[:], disown_sb[:, pid:pid + 1],
                            None, alu.mult)
                        nc.vector.tensor_add(o_sb[:], o_sb[:], bias_sb[:])
                        nc.sync.dma_start(
                            out_d[pid * 128:(pid + 1) * 128, :], o_sb[:])

    nc.compile()
    return nc


_NC_CACHE = {}


def _run(inputs, trace=False):
    from concourse.bass_utils import run_bass_kernel_spmd

    x = np.asarray(inputs["x"], np.float32)
    cfg, shared, cores, target_rows = prep(
        x, inputs["edge_index"], inputs["f_disc_orig"], inputs["fluxes"])
    w2c, w2d, bias2 = _shared_weights(
        inputs["W_conc"], inputs["W_disc"], inputs["bias"])

    in_maps = []
    for core in range(N_CORES):
        m = dict(shared)
        m.update(cores[core])
        m["w2c"] = w2c
        m["w2d"] = w2d
        m["bias2"] = bias2
        in_maps.append(m)

    if cfg not in _NC_CACHE:
        _NC_CACHE[cfg] = build_nc(cfg)
    nc = _NC_CACHE[cfg]

    res = run_bass_kernel_spmd(nc, in_maps, list(range(N_CORES)),
                               trace=trace)
    out = np.zeros((BATCH, N_NODES, C), np.float32)
    for core in range(N_CORES):
        r = np.asarray(res.results[core]["out"], np.float32)  # [6400, 128]
        tr = target_rows[core]
        valid = tr >= 0
        out[0, tr[valid]] = r[valid, :C]
        out[1, tr[valid]] = r[valid, C:]
    return out, res


def kernel(x, edge_index, f_disc_orig, fluxes, W_conc, W_disc, bias):
    out, _ = _run(dict(x=x, edge_index=edge_index, f_disc_orig=f_disc_orig,
                       fluxes=fluxes, W_conc=W_conc, W_disc=W_disc,
                       bias=bias))
    return out


def profile_run(inputs):
    out, res = _run(inputs, trace=True)
    return res.exec_time_ns


# revision 20
# speedup vs baseline: 2.3903x; 2.3903x over previous
"""Trainium2 Bass kernel for nn_ReaReaConv (GCN-style message passing with
dynamic edge gating) — bf16 redesign.

Math (per batch b):
    deg[n]   = in-degree(n) + 1 (self loop);  dis = rsqrt(deg)
    f_e      = keep*fdo + (1-keep)*(1-fdo), keep = sigmoid(2*flux[src]*flux[tgt])
    out[t]   = dis_t * ( Wc @ (T-V)[t] + Wd @ V[t] ) + bias
    T[t]     = sum_{e->t} dis_src * x[src_e]          (self loop: f=0 edge)
    V[t]     = sum_{e->t} dis_src * f_e * x[src_e]

Design:
 - Targets are bin-packed (host, integer-only) into 800 tiles of 64 targets
   each, balancing per-tile A/B edge counts so chunk capacities stay tight.
   Output rows are permuted back on the host.
 - Gather tables are bf16 with batch-interleaved rows (x[n] = [f0b0, f0b1,
   f1b0, f1b1, ...], 256B rows) split A/B at 32768 for int16 indices.
 - Per 4-tile group: 2 dma_gathers (A/B tables), one merged is_equal builds
   all chunks' one-hots (pair-duplicated tl/iota for DVE 2x bf16 mode), one
   merged multiply scales by dis_src, one merged multiply builds f-scaled
   V-operands (f0/f1 pairs ride the batch-interleave).
 - Matmuls: one-hot chunk is the stationary operand ([128 edges, 64 tgts]);
   raw x and f-scaled x are the moving operands, accumulating T/V in a
   [128, 128] PSUM tile holding a PAIR of 64-target tiles via col strips
   (tile_position=(0, 0|64)).
 - Epilogue per pair: um/vm -> PE transpose -> projection with interleaved
   block weights W2 (also de-interleaves batches) -> dis_tgt scale + bias.
 - Trailing pad slots use index -1 (Q7 trims them: no DMA descriptors).
"""

import heapq

import numpy as np
import ml_dtypes

BF16 = ml_dtypes.bfloat16

# -------------------- problem constants --------------------
N_NODES = 50000
N_EDGES = 1600000
BATCH = 2
C = 64            # per-batch channels
IC = 2 * C        # interleaved feature width (both batches)
N_CORES = 8
T = 64            # targets per tile
TPG = 4           # tiles per gather group
NT_CORE = 100     # tiles per core (incl. pad tiles)
NG = NT_CORE // TPG
NPAIR = NT_CORE // 2
N_TILES = N_CORES * NT_CORE   # 800
CHUNK = 128
SPLIT = 32768
SELF_FLUX = 30.0  # sigmoid(2*30*30)==1.0 -> f==0 for self-loop edges
PAD_NEG1 = False  # -1 trailing-pad idxs fault on deployed HW ucode; keep 0-pads


class Cfg(tuple):
    """(capa, capb) chunk capacities per tile for the A/B tables."""
    @property
    def capa(self):
        return self[0]

    @property
    def capb(self):
        return self[1]

    @property
    def ct(self):
        return self[0] + self[1]

    @property
    def ctg(self):          # chunks per group
        return TPG * self.ct

    @property
    def ctn(self):          # chunks per core
        return NG * self.ctg

    @property
    def na(self):
        return min(SPLIT, N_NODES)

    @property
    def nb(self):
        return N_NODES - self.na


# -------------------- host prep (indices / layout only) --------------------

def _wrap16(idx_flat):
    """dma_gather index layout: [128, n/16] int16, idx[p, s] = flat[s*16+p],
    replicated across the 8 gpsimd cores (partition blocks of 16)."""
    n = len(idx_flat)
    assert n % 16 == 0
    w = np.asarray(idx_flat, np.int16).reshape(n // 16, 16).T  # [16, n/16]
    return np.tile(w, (8, 1))  # [128, n/16]


def _assign_tiles(degA, degB):
    """Greedy 2D bin-packing of targets into N_TILES tiles of <=T targets,
    balancing per-tile A and B edge counts. Integer/layout work only."""
    deg = degA + degB
    order = np.argsort(-deg, kind="stable")
    avgA = max(float(degA.sum()) / N_TILES, 1.0)
    avgB = max(float(degB.sum()) / N_TILES, 1.0)
    loadA = np.zeros(N_TILES, np.int64)
    loadB = np.zeros(N_TILES, np.int64)
    counts = np.zeros(N_TILES, np.int32)
    heap = [(0.0, i) for i in range(N_TILES)]
    heapq.heapify(heap)
    tile_of = np.empty(N_NODES, np.int32)
    slot_of = np.empty(N_NODES, np.int32)
    for t in order:
        while True:
            key, i = heapq.heappop(heap)
            if counts[i] < T:
                break
        tile_of[t] = i
        slot_of[t] = counts[i]
        counts[i] += 1
        loadA[i] += degA[t]
        loadB[i] += degB[t]
        if counts[i] < T:
            heapq.heappush(
                heap, (max(loadA[i] / avgA, loadB[i] / avgB), i))
    return tile_of, slot_of


def prep(x, edge_index, f_disc_orig, fluxes):
    """Returns (cfg, shared dict, per-core dicts, target_rows). Integer /
    index / layout / dtype work only — no floating-point arithmetic."""
    n = N_NODES
    src0 = np.asarray(edge_index[0]).astype(np.int64)
    tgt0 = np.asarray(edge_index[1]).astype(np.int64)
    x = np.asarray(x, np.float32)
    fdo_in = np.asarray(f_disc_orig, np.float32)
    fluxes = np.asarray(fluxes, np.float32)

    deg = (np.bincount(tgt0, minlength=n) + 1).astype(np.int64)

    # per-target A/B in-edge counts (self loop included)
    isA0 = src0 < SPLIT
    degA = np.bincount(tgt0[isA0], minlength=n)
    degA += (np.arange(n) < SPLIT)
    degB = deg - degA

    tile_of, slot_of = _assign_tiles(degA, degB)

    # all edges incl self loops
    loops = np.arange(n, dtype=np.int64)
    src_all = np.concatenate([src0, loops])
    tgt_all = np.concatenate([tgt0, loops])
    sf = np.full(n, SELF_FLUX, np.float32)
    per_edge = np.stack([
        np.concatenate([fdo_in, np.zeros(n, np.float32)]),
        np.concatenate([fluxes[0][src0], sf]),
        np.concatenate([fluxes[1][src0], sf]),
        np.concatenate([fluxes[0][tgt0], sf]),
        np.concatenate([fluxes[1][tgt0], sf]),
        deg[src_all].astype(np.float32),
    ])  # [6, E+N]: fdo, fs0, fs1, ft0, ft1, degs

    tid = tile_of[tgt_all].astype(np.int64)
    table = (src_all >= SPLIT).astype(np.int64)
    bucket = tid * 2 + table
    order = np.argsort(bucket, kind="stable")
    counts = np.bincount(bucket, minlength=N_TILES * 2)
    starts = np.concatenate([[0], np.cumsum(counts)])
    rank = np.arange(len(order)) - np.repeat(starts[:-1], counts)

    capa = int(-(-counts[0::2].max() // CHUNK))
    capb = int(-(-counts[1::2].max() // CHUNK))
    cfg = Cfg((capa, capb))
    ct, ctg, ctn = cfg.ct, cfg.ctg, cfg.ctn

    # within-core chunk base per (tile, table)
    t_local = np.arange(N_TILES) % NT_CORE
    g_of = t_local // TPG
    k_of = t_local % TPG
    baseA = g_of * ctg + k_of * capa
    baseB = g_of * ctg + TPG * capa + k_of * capb
    base_chunk = np.empty(N_TILES * 2, np.int64)
    base_chunk[0::2] = baseA
    base_chunk[1::2] = baseB

    eo = order
    cc = base_chunk[bucket[eo]] + rank // CHUNK     # within-core chunk
    pp = rank % CHUNK                               # partition
    co = tid[eo] // NT_CORE                         # core

    tl = np.full((N_CORES, 128, ctn), -1.0, np.float32)
    meta = np.zeros((N_CORES, 6, 128, ctn), np.float32)
    meta[:, 1:5] = SELF_FLUX    # pad fs/ft -> keep=1, fdo=0 -> f=0
    meta[:, 5] = 1.0            # pad deg_src = 1
    idxflat = np.zeros((N_CORES, ctn * CHUNK), np.int64)
    realm = np.zeros((N_CORES, ctn * CHUNK), bool)

    tl[co, pp, cc] = slot_of[tgt_all[eo]].astype(np.float32)
    for j in range(6):
        meta[co, j, pp, cc] = per_edge[j][eo]
    src_adj = src_all[eo] - table[eo] * cfg.na
    idxflat[co, cc * CHUNK + pp] = src_adj
    realm[co, cc * CHUNK + pp] = True

    # trailing -1 per gather call; wrap16 per call
    cores = []
    for core in range(N_CORES):
        blocks = []
        for g in range(NG):
            a0 = (g * ctg) * CHUNK
            a1 = (g * ctg + TPG * capa) * CHUNK
            b1 = ((g + 1) * ctg) * CHUNK
            for (s, e) in ((a0, a1), (a1, b1)):
                seg = idxflat[core, s:e].copy()
                if PAD_NEG1:
                    rm = realm[core, s:e]
                    nz = np.nonzero(rm)[0]
                    last = nz[-1] if len(nz) else -1
                    seg[last + 1:] = -1
                blocks.append(_wrap16(seg))
        idx16 = np.concatenate(blocks, axis=1)

        tl2 = np.repeat(tl[core], 2, axis=1).astype(BF16)  # [128, ctn*2]
        d = {
            "idx16": idx16,
            "tl2": np.ascontiguousarray(tl2),
        }
        for j, nm in enumerate(["fdo", "fs0", "fs1", "ft0", "ft1", "degs"]):
            d[nm] = np.ascontiguousarray(meta[core, j])
        cores.append(d)

    # target id per output row: row = tile_local*T + slot
    target_rows = np.full((N_CORES, NT_CORE * T), -1, np.int64)
    target_rows[tile_of // NT_CORE, (tile_of % NT_CORE) * T + slot_of] = \
        np.arange(n)

    # degown: [128, NPAIR], partition = row within pair
    for core in range(N_CORES):
        tr = target_rows[core]
        dg = np.ones(NT_CORE * T, np.float32)
        valid = tr >= 0
        dg[valid] = deg[tr[valid]].astype(np.float32)
        cores[core]["degown"] = np.ascontiguousarray(
            dg.reshape(NPAIR, 128).T)

    # shared gather tables: batch-interleaved bf16 rows
    xp = np.empty((n, IC), np.float32)
    xp[:, 0::2] = x[0]
    xp[:, 1::2] = x[1]
    xp16 = xp.astype(BF16)
    shared = {
        "xpa": np.ascontiguousarray(xp16[:cfg.na]),
        "xpb": np.ascontiguousarray(xp16[cfg.na:]),
        "iota2": np.tile(np.arange(T, dtype=np.float32).astype(BF16),
                         (128, 1)),
        "ident": np.eye(128, dtype=np.float32).astype(BF16),
    }
    return cfg, shared, cores, target_rows


def _shared_weights(W_conc, W_disc, bias):
    """W2[2j+b, o+64b] = W[o, j]; bias2 replicated (fp32)."""
    Wc = np.asarray(W_conc, np.float32)
    Wd = np.asarray(W_disc, np.float32)
    w2c = np.zeros((128, IC), np.float32)
    w2d = np.zeros((128, IC), np.float32)
    j = np.arange(C)
    o = np.arange(C)
    for b in range(2):
        w2c[np.ix_(2 * j + b, o + C * b)] = Wc.T
        w2d[np.ix_(2 * j + b, o + C * b)] = Wd.T
    bias2 = np.zeros((128, IC), np.float32)
    bias2[:, :C] = np.asarray(bias, np.float32)[None, :]
    bias2[:, C:] = np.asarray(bias, np.float32)[None, :]
    return w2c.astype(BF16), w2d.astype(BF16), bias2


# -------------------- device program --------------------

def build_nc(cfg: Cfg, qmap=None):
    """qmap: {(group, table): queue_num}; None = all queue 0."""
    import concourse.bass as bass  # noqa: F401
    import concourse.tile as tile
    from concourse import bacc, mybir

    if qmap is None:
        qmap = {}

    dt = mybir.dt
    act = mybir.ActivationFunctionType
    alu = mybir.AluOpType

    capa, capb, ct, ctg, ctn = cfg.capa, cfg.capb, cfg.ct, cfg.ctg, cfg.ctn
    cag, cbg = TPG * capa, TPG * capb

    nc = bacc.Bacc("TRN2", target_bir_lowering=False, debug=False,
                   num_swdge_queues=4)

    xpa = nc.dram_tensor("xpa", [cfg.na, IC], dt.bfloat16, kind="ExternalInput")
    xpb = nc.dram_tensor("xpb", [cfg.nb, IC], dt.bfloat16, kind="ExternalInput")
    idx16_d = nc.dram_tensor("idx16", [128, ctn * 8], dt.int16,
                             kind="ExternalInput")
    tl2_d = nc.dram_tensor("tl2", [128, ctn * 2], dt.bfloat16,
                           kind="ExternalInput")
    fdo_d = nc.dram_tensor("fdo", [128, ctn], dt.float32, kind="ExternalInput")
    fs0_d = nc.dram_tensor("fs0", [128, ctn], dt.float32, kind="ExternalInput")
    fs1_d = nc.dram_tensor("fs1", [128, ctn], dt.float32, kind="ExternalInput")
    ft0_d = nc.dram_tensor("ft0", [128, ctn], dt.float32, kind="ExternalInput")
    ft1_d = nc.dram_tensor("ft1", [128, ctn], dt.float32, kind="ExternalInput")
    degs_d = nc.dram_tensor("degs", [128, ctn], dt.float32,
                            kind="ExternalInput")
    degown_d = nc.dram_tensor("degown", [128, NPAIR], dt.float32,
                              kind="ExternalInput")
    iota2_d = nc.dram_tensor("iota2", [128, T], dt.bfloat16,
                             kind="ExternalInput")
    ident_d = nc.dram_tensor("ident", [128, 128], dt.bfloat16,
                             kind="ExternalInput")
    w2c_d = nc.dram_tensor("w2c", [128, IC], dt.bfloat16, kind="ExternalInput")
    w2d_d = nc.dram_tensor("w2d", [128, IC], dt.bfloat16, kind="ExternalInput")
    bias2_d = nc.dram_tensor("bias2", [128, IC], dt.float32,
                             kind="ExternalInput")
    out_d = nc.dram_tensor("out", [NT_CORE * T, IC], dt.float32,
                           kind="ExternalOutput")

    with tile.TileContext(nc) as tc:
        with (
            tc.tile_pool(name="const", bufs=1) as constp,
            tc.tile_pool(name="res", bufs=1) as resp,
        ):
            iota_sb = constp.tile([128, T], dt.bfloat16)
            nc.sync.dma_start(iota_sb[:], iota2_d[:, :])
            ident_sb = constp.tile([128, 128], dt.bfloat16)
            nc.sync.dma_start(ident_sb[:], ident_d[:, :])
            w2c_sb = constp.tile([128, IC], dt.bfloat16)
            nc.sync.dma_start(w2c_sb[:], w2c_d[:, :])
            w2d_sb = constp.tile([128, IC], dt.bfloat16)
            nc.sync.dma_start(w2d_sb[:], w2d_d[:, :])
            bias_sb = constp.tile([128, IC], dt.float32)
            nc.sync.dma_start(bias_sb[:], bias2_d[:, :])

            idx_sb = resp.tile([128, ctn * 8], dt.int16)
            nc.sync.dma_start(idx_sb[:], idx16_d[:, :])
            tl2_sb = resp.tile([128, ctn * 2], dt.bfloat16)
            nc.sync.dma_start(tl2_sb[:], tl2_d[:, :])
            g2_sb = resp.tile([128, ctn * 2], dt.bfloat16)
            f01_sb = resp.tile([128, ctn * 2], dt.bfloat16)
            disown_sb = resp.tile([128, NPAIR], dt.float32)
            nc.sync.dma_start(disown_sb[:], degown_d[:, :])
            nc.vector.reciprocal(disown_sb[:], disown_sb[:])
            nc.scalar.activation(disown_sb[:], disown_sb[:], act.Sqrt)

            # ---- prepass: g2 (dis_src pairs) and f01 (f0/f1 pairs) ----
            with tc.tile_pool(name="pp", bufs=1) as ppp:
                g_sb = ppp.tile([128, ctn], dt.float32)
                nc.sync.dma_start(g_sb[:], degs_d[:, :])
                nc.vector.reciprocal(g_sb[:], g_sb[:])
                nc.scalar.activation(g_sb[:], g_sb[:], act.Sqrt)
                g2v = g2_sb[:].rearrange("p (c two) -> p c two", two=2)
                nc.vector.tensor_copy(out=g2v[:, :, 0], in_=g_sb[:])
                nc.vector.tensor_copy(out=g2v[:, :, 1], in_=g_sb[:])

                fdo_sb = ppp.tile([128, ctn], dt.float32)
                nc.sync.dma_start(fdo_sb[:], fdo_d[:, :])
                c1 = ppp.tile([128, ctn], dt.float32)
                nc.vector.tensor_scalar(
                    c1[:], fdo_sb[:], 2.0, -1.0, alu.mult, alu.add)
                c0 = ppp.tile([128, ctn], dt.float32)
                nc.vector.tensor_scalar(
                    c0[:], fdo_sb[:], -1.0, 1.0, alu.mult, alu.add)
                f01v = f01_sb[:].rearrange("p (c two) -> p c two", two=2)
                for b, (fsd, ftd) in enumerate(((fs0_d, ft0_d),
                                                (fs1_d, ft1_d))):
                    fs_sb = ppp.tile([128, ctn], dt.float32, tag="fs")
                    nc.sync.dma_start(fs_sb[:], fsd[:, :])
                    ft_sb = ppp.tile([128, ctn], dt.float32, tag="ft")
                    nc.sync.dma_start(ft_sb[:], ftd[:, :])
                    nc.vector.tensor_mul(fs_sb[:], fs_sb[:], ft_sb[:])
                    nc.scalar.activation(
                        ft_sb[:], fs_sb[:], act.Sigmoid, scale=2.0)
                    nc.vector.tensor_mul(ft_sb[:], ft_sb[:], c1[:])
                    nc.vector.tensor_add(ft_sb[:], ft_sb[:], c0[:])
                    nc.vector.tensor_copy(out=f01v[:, :, b], in_=ft_sb[:])

            # ---- main loop over gather groups ----
            with (
                tc.tile_pool(name="xg", bufs=4) as xgp,
                tc.tile_pool(name="oh", bufs=2) as ohp,
                tc.tile_pool(name="wv", bufs=2) as wvp,
                tc.tile_pool(name="uv", bufs=2) as uvp,
                tc.tile_pool(name="uvt", bufs=2) as uvtp,
                tc.tile_pool(name="outp", bufs=2) as outsp,
                tc.tile_pool(name="ps_tv", bufs=2, space="PSUM") as pstv,
                tc.tile_pool(name="ps_tr", bufs=2, space="PSUM") as pstr,
                tc.tile_pool(name="ps_o", bufs=2, space="PSUM") as pso,
            ):
                tl2v = tl2_sb[:].rearrange("p (c two) -> p c two", two=2)
                g2v = g2_sb[:].rearrange("p (c two) -> p c two", two=2)
                f01v = f01_sb[:].rearrange("p (c two) -> p c two", two=2)
                iota4 = (iota_sb[:]
                         .rearrange("p (a b) -> p a b", b=2)
                         .unsqueeze(1))
                for g in range(NG):
                    xg = xgp.tile([128, ctg * IC], dt.bfloat16, tag="xg")
                    if g < 4:
                        nc.vector.memset(xg[:], 0.0)
                    xg3 = xg[:].rearrange("p (c r) -> p c r", r=IC)
                    ib = g * ctg * 8
                    nc.gpsimd.dma_gather(
                        xg3[:, 0:cag], xpa[:, :],
                        idx_sb[:, ib: ib + cag * 8],
                        cag * CHUNK, cag * CHUNK, IC,
                        single_packet=False, queue_num=qmap.get((g, 0), 0),
                    )
                    nc.gpsimd.dma_gather(
                        xg3[:, cag:ctg], xpb[:, :],
                        idx_sb[:, ib + cag * 8: ib + ctg * 8],
                        cbg * CHUNK, cbg * CHUNK, IC,
                        single_packet=False, queue_num=qmap.get((g, 1), 0),
                    )

                    gc = slice(g * ctg, (g + 1) * ctg)
                    # one-hot: is_equal then *dis_src (pair-packed, 2x mode)
                    o_all = ohp.tile([128, ctg * T], dt.bfloat16, tag="oh")
                    o4 = o_all[:].rearrange("p (c a b) -> p c a b", a=T // 2,
                                            b=2)
                    nc.vector.tensor_tensor(
                        o4,
                        tl2v[:, gc, :].unsqueeze(2)
                        .to_broadcast([128, ctg, T // 2, 2]),
                        iota4.to_broadcast([128, ctg, T // 2, 2]),
                        alu.is_equal,
                    )
                    nc.vector.tensor_tensor(
                        o4, o4,
                        g2v[:, gc, :].unsqueeze(2)
                        .to_broadcast([128, ctg, T // 2, 2]),
                        alu.mult,
                    )
                    # f-scaled V operand (f0/f1 pairs ride batch interleave)
                    wv = wvp.tile([128, ctg * IC], dt.bfloat16, tag="wv")
                    wv4 = wv[:].rearrange("p (c a b) -> p c a b", a=C, b=2)
                    xg4 = xg[:].rearrange("p (c a b) -> p c a b", a=C, b=2)
                    nc.vector.tensor_tensor(
                        wv4, xg4,
                        f01v[:, gc, :].unsqueeze(2)
                        .to_broadcast([128, ctg, C, 2]),
                        alu.mult,
                    )

                    o3 = o_all[:].rearrange("p (c t) -> p c t", t=T)
                    wv3 = wv[:].rearrange("p (c r) -> p c r", r=IC)
                    for q in range(2):
                        t_tile = pstv.tile([128, IC], dt.float32,
                                           tag="t_ps", name="t_tile")
                        v_tile = pstv.tile([128, IC], dt.float32,
                                           tag="v_ps", name="v_tile")
                        t_ps = t_tile[:, :]
                        v_ps = v_tile[:, :]
                        for ci in range(ct):
                            for s in range(2):
                                k = 2 * q + s
                                if ci < capa:
                                    c = k * capa + ci
                                else:
                                    c = cag + k * capb + (ci - capa)
                                lhs = o3[:, c, :]
                                rows = slice(64 * s, 64 * s + 64)
                                nc.tensor.matmul(
                                    out=t_ps[rows, :], lhsT=lhs,
                                    rhs=xg3[:, c, :],
                                    start=(ci == 0), stop=(ci == ct - 1),
                                    tile_position=(0, 64 * s),
                                    skip_group_check=True,
                                )
                                nc.tensor.matmul(
                                    out=v_ps[rows, :], lhsT=lhs,
                                    rhs=wv3[:, c, :],
                                    start=(ci == 0), stop=(ci == ct - 1),
                                    tile_position=(0, 64 * s),
                                    skip_group_check=True,
                                )

                        # epilogue for this pair of tiles
                        pid = g * 2 + q
                        vm = uvp.tile([128, IC], dt.bfloat16, tag="vm")
                        nc.vector.tensor_copy(out=vm[:], in_=v_ps)
                        um = uvp.tile([128, IC], dt.bfloat16, tag="um")
                        nc.vector.tensor_tensor(
                            um[:], t_ps, vm[:], alu.subtract)
                        tr_ps = pstr.tile([128, 256], dt.bfloat16,
                                          tag="tr")
                        nc.tensor.transpose(
                            tr_ps[:, 0:128], um[:], ident_sb[:])
                        nc.tensor.transpose(
                            tr_ps[:, 128:256], vm[:], ident_sb[:])
                        uvt = uvtp.tile([128, 256], dt.bfloat16, tag="uvt")
                        nc.vector.tensor_copy(out=uvt[:], in_=tr_ps[:])
                        umt = uvt[:, 0:128]
                        vmt = uvt[:, 128:256]
                        op_ps = pso.tile([128, IC], dt.float32, tag="op")
                        nc.tensor.matmul(
                            out=op_ps[:], lhsT=umt, rhs=w2c_sb[:],
                            start=True, stop=False)
                        nc.tensor.matmul(
                            out=op_ps[:], lhsT=vmt, rhs=w2d_sb[:],
                            start=False, stop=True)
                        o_sb = outsp.tile([128, IC], dt.float32, tag="os")
                        nc.vector.tensor_scalar(
                            o_sb[:], op_ps[:], disown_sb[:, pid:pid + 1],
                            None, alu.mult)
                        nc.vector.tensor_add(o_sb[:], o_sb[:], bias_sb[:])
                        nc.sync.dma_start(
                            out_d[pid * 128:(pid + 1) * 128, :], o_sb[:])

    nc.compile()
    return nc


_NC_CACHE = {}


def _gather_sched(nc):
    """[(sched_pos, idx_ap_offset, queue_num)] for gathers in scheduled
    order."""
    out = []
    pos = 0
    for blk in nc.m.functions[0].blocks:
        for inst in blk.instructions:
            if type(inst).__name__ == "InstDMAGatherAnt":
                off = None
                for arg in inst.ins:
                    ap = getattr(arg, "ap", None)
                    if ap is not None and getattr(arg, "dtype", None) is not None:
                        if "int16" in str(arg.dtype):
                            off = arg.offset
                out.append((pos, off, inst.queue_num))
                pos += 1
    return out


def _build_nc_queued(cfg):
    """Two-pass build: discover the tile scheduler's gather order, then
    assign SWDGE queues so each DMASW sem lane (sched order % 8) sees a
    single queue (sched order % 4)."""
    nc1 = build_nc(cfg)
    sched = _gather_sched(nc1)
    if any(off is None for _, off, _ in sched):
        return nc1
    by_off = sorted(sched, key=lambda t: t[1])
    qmap = {}
    for rank, (pos, _off, _q) in enumerate(by_off):
        qmap[(rank // 2, rank % 2)] = pos % 4
    nc2 = build_nc(cfg, qmap)
    sched2 = _gather_sched(nc2)
    by_off2 = sorted(sched2, key=lambda t: t[1])
    for rank, (pos, _off, q) in enumerate(by_off2):
        if pos % 4 != q or by_off[rank][0] != pos:
            # scheduler order changed between passes; single queue is safe
            return nc1
    return nc2


def _run(inputs, trace=False):
    from concourse.bass_utils import run_bass_kernel_spmd

    x = np.asarray(inputs["x"], np.float32)
    cfg, shared, cores, target_rows = prep(
        x, inputs["edge_index"], inputs["f_disc_orig"], inputs["fluxes"])
    w2c, w2d, bias2 = _shared_weights(
        inputs["W_conc"], inputs["W_disc"], inputs["bias"])

    in_maps = []
    for core in range(N_CORES):
        m = dict(shared)
        m.update(cores[core])
        m["w2c"] = w2c
        m["w2d"] = w2d
        m["bias2"] = bias2
        in_maps.append(m)

    if cfg not in _NC_CACHE:
        _NC_CACHE[cfg] = _build_nc_queued(cfg)
    nc = _NC_CACHE[cfg]

    res = run_bass_kernel_spmd(nc, in_maps, list(range(N_CORES)),
                               trace=trace)
    out = np.zeros((BATCH, N_NODES, C), np.float32)
    for core in range(N_CORES):
        r = np.asarray(res.results[core]["out"], np.float32)  # [6400, 128]
        tr = target_rows[core]
        valid = tr >= 0
        out[0, tr[valid]] = r[valid, :C]
        out[1, tr[valid]] = r[valid, C:]
    return out, res


def kernel(x, edge_index, f_disc_orig, fluxes, W_conc, W_disc, bias):
    out, _ = _run(dict(x=x, edge_index=edge_index, f_disc_orig=f_disc_orig,
                       fluxes=fluxes, W_conc=W_conc, W_disc=W_disc,
                       bias=bias))
    return out


def profile_run(inputs):
    out, res = _run(inputs, trace=True)
    return res.exec_time_ns
